# revision 5
# baseline (speedup 1.0000x reference)
"""CQVAE loss kernel for Trainium2, data-parallel over batch on 8 NeuronCores.

loss = kld(qy) + mse(gather(rzs), zs[:, :Sg]) + bias(best, best_gt)
       + bias(gather(pts), gts)
where bias(p, g) = mse(p, g) + 10 * mse(p[..., MARK, :], g[..., MARK, :]).

v2 design (per core, 16 of 128 batches):
- Host packs comb rows [rzs | pts(mark-cols-first)] and a NEGATED target
  [zs | gts(mark-cols-first)], both fp8e4m3 (tolerance 2e-2 >> fp8 noise).
- The target loads as 4 direct chunks; the mapping-gather runs as 4
  multi-descriptor indirect DMAs (512 rows each, ~1.2us emission instead
  of ~27us for row-per-op) with compute_op=add, so SBUF ends up holding
  d = comb_gathered - tgt with no vector subtract pass at all.
- d**2 reduction is split across three engines: PE computes most of the
  rzs region via accumulated 128x128 self-matmuls into one PSUM tile
  (host sums its diagonal), Act squares a middle column slice, DVE does
  the remainder + pts + mark + kld + best accumulators.
- Landmark / denominator weights fold into host-side column scaling
  (best) or separate accumulators (pts marks are permuted to the front
  of the pts block so one contiguous accumulator covers them).
"""

import sys

import numpy as np

try:
    import concourse  # noqa: F401
except ImportError:  # pragma: no cover
    sys.path.insert(0, "/opt/trn_rl_repo")

import ml_dtypes

import concourse.bass as bass  # noqa: F401
import concourse.mybir as mybir
import concourse.tile as tile
from concourse import bacc
from concourse.bass_utils import run_bass_kernel_spmd

F32 = mybir.dt.float32
BF16 = mybir.dt.bfloat16
F8 = mybir.dt.float8e4
I32 = mybir.dt.int32
AX = mybir.AxisListType
OP = mybir.AluOpType
ACTF = mybir.ActivationFunctionType

NCORES = 8
B, S, SG, D, P, V = 128, 256, 128, 1024, 118, 64
BL = B // NCORES  # batches per core
P2 = 2 * P  # 236 point floats per row
CW = D + P2  # 1260 combined row width
MARK = (0, 29, 88, 117)
NMARK2 = 2 * len(MARK)  # 8 mark columns
EPS = 1e-20
ALPHA = 10.0

NCH = 4  # gather/compute chunks
KC = BL // NCH  # 4 slots per chunk
CHW = KC * CW  # 5040 cols per chunk tile

# zs-region (1024 cols/slot) engine split: PE | Act | DVE
NPE = 5  # 128-col matmul pairs per slot on the tensor engine
PECOL = 128 * NPE  # 640
ACOL = 220  # activation-engine slice
DV0 = PECOL + ACOL  # 860; DVE covers [DV0:1024)
DVW = D - DV0  # 164

QN = BL * S // 128  # 32 qy rows per partition
QCOLS = QN * V  # 2048

NSTAT = 20
C_AZS = 0  # 4 cols: Act zs accums
C_VZS = 4  # 4 cols: DVE zs accums
C_PTS = 8  # 4 cols: pts-all accums
C_MRK = 12  # 4 cols: pts mark accums
C_KLD = 16
C_BEST = 17

MW = float(np.sqrt(1.0 + ALPHA * P2 / NMARK2))  # 17.2047 best-mark fold

_module = None
last_results = None  # BassKernelResults of the most recent run (for profiling)


def _build_module():
    nc = bacc.Bacc()

    # comb row r=b*S+s : concat(rzs[b,s], pts_perm[b,s]) fp8
    comb = nc.dram_tensor("comb", [BL * S, CW], F8, kind="ExternalInput")
    # tgt row p*16+k : -concat(zs[b,i], gts_perm[b,i]), b=p//8, i=16*(p%8)+k
    tgt = nc.dram_tensor("tgt", [128 * BL, CW], F8, kind="ExternalInput")
    qy = nc.dram_tensor("qy", [BL * S, V], BF16, kind="ExternalInput")
    best = nc.dram_tensor("best", [BL, P2], F32, kind="ExternalInput")
    best_gt = nc.dram_tensor("best_gt", [BL, P2], F32, kind="ExternalInput")
    # idx[p, k] = (p//8)*S + mapping[p//8, 16*(p%8) + k]
    idx2 = nc.dram_tensor("idx2", [128, BL], I32, kind="ExternalInput")
    out = nc.dram_tensor("out", [128, NSTAT], F32, kind="ExternalOutput")
    out2 = nc.dram_tensor("out2", [128, 128], F32, kind="ExternalOutput")

    with tile.TileContext(nc) as tc:
        with tc.tile_pool(name="cst", bufs=1) as cst, tc.psum_pool(
            name="ps", bufs=1
        ) as psp:
            idx_t = cst.tile([128, BL], I32)
            nc.sync.dma_start(idx_t[:], idx2[:])

            stats = cst.tile([128, NSTAT], F32)
            nc.vector.memset(stats[:], 0.0)
            ebias = cst.tile([128, 1], F32)
            nc.vector.memset(ebias[:], float(V) * EPS)

            # qy on the scalar HWDGE queue (early, feeds Ln)
            qy_t = cst.tile([128, QCOLS], BF16)
            nc.scalar.dma_start(
                qy_t[:], qy[:].rearrange("(p n) v -> p (n v)", n=QN)
            )

            # tgt chunks on the sync HWDGE queue
            tgt_r = tgt[:].rearrange("(p k) c -> p (k c)", k=BL)
            ch = []
            for c in range(NCH):
                t = cst.tile([128, CHW], F8, tag=f"ch{c}", name=f"ch{c}")
                nc.sync.dma_start(t[:], tgt_r[:, c * CHW : (c + 1) * CHW])
                ch.append(t)
            bt = cst.tile([BL, P2], F32)
            nc.sync.dma_start(bt[:], best[:])
            bgt = cst.tile([BL, P2], F32)
            nc.sync.dma_start(bgt[:], best_gt[:])

            # gathers: per-slot ops (the SWDGE ucode honors only a [128,1]
            # offset column), accumulating onto the negated target so the
            # tile holds comb_g - [zs|gts] with no vector subtract pass
            for c in range(NCH):
                for k in range(KC):
                    nc.gpsimd.indirect_dma_start(
                        out=ch[c][:, k * CW : (k + 1) * CW],
                        out_offset=None,
                        in_=comb[:],
                        in_offset=bass.IndirectOffsetOnAxis(
                            ap=idx_t[:, c * KC + k : c * KC + k + 1], axis=0
                        ),
                        compute_op=OP.add,
                    )

            # ---- PE: accumulated self-matmuls over zs cols [0:PECOL) -----
            cps = psp.tile([128, 128], F32)
            npair = NCH * KC * NPE
            i = 0
            for c in range(NCH):
                for k in range(KC):
                    for j in range(NPE):
                        o = k * CW + j * 128
                        blk = ch[c][:, o : o + 128]
                        nc.tensor.matmul(
                            cps[:], blk, blk,
                            start=(i == 0), stop=(i == npair - 1),
                        )
                        i += 1

            # ---- Act: Ln(qy) then zs middle slices ------------------------
            lg = cst.tile([128, QCOLS], BF16)
            nc.scalar.activation(lg[:], qy_t[:], ACTF.Ln, bias=ebias[:], scale=float(V))
            scr_a = cst.tile([128, KC * ACOL], BF16)
            sa3 = scr_a[:].rearrange("p (k w) -> p k w", w=ACOL)
            for c in range(NCH):
                c3 = ch[c][:].rearrange("p (k w) -> p k w", w=CW)
                nc.scalar.activation(
                    sa3, c3[:, :, PECOL : PECOL + ACOL], ACTF.Square,
                    accum_out=stats[:, C_AZS + c : C_AZS + c + 1],
                )

            # ---- DVE: kld, zs tail, pts, marks, best ----------------------
            scr_k = cst.tile([128, QCOLS], BF16)
            scr_v = cst.tile([128, 2048], BF16)
            sv_z = scr_v[:, : KC * DVW].rearrange("p (k w) -> p k w", w=DVW)
            sv_p = scr_v[:, KC * DVW : KC * DVW + KC * P2].rearrange(
                "p (k w) -> p k w", w=P2
            )
            sv_m = scr_v[:, 1900 : 1900 + KC * NMARK2].rearrange(
                "p (k w) -> p k w", w=NMARK2
            )

            def sq_acc(out_ap, in_ap, acc):
                nc.vector.scalar_tensor_tensor(
                    out=out_ap, in0=in_ap, scalar=0.0, in1=in_ap,
                    op0=OP.subtract, op1=OP.mult, accum_out=acc,
                )

            # kld: sum q * (ln(V*q + V*eps)) = sum q*(ln q - ln(1/V))
            nc.vector.scalar_tensor_tensor(
                out=scr_k[:], in0=lg[:], scalar=0.0, in1=qy_t[:],
                op0=OP.subtract, op1=OP.mult,
                accum_out=stats[:, C_KLD : C_KLD + 1],
            )
            for c in range(NCH):
                c3 = ch[c][:].rearrange("p (k w) -> p k w", w=CW)
                sq_acc(sv_z, c3[:, :, DV0:D], stats[:, C_VZS + c : C_VZS + c + 1])
                sq_acc(sv_p, c3[:, :, D:CW], stats[:, C_PTS + c : C_PTS + c + 1])
                sq_acc(
                    sv_m, c3[:, :, D : D + NMARK2],
                    stats[:, C_MRK + c : C_MRK + c + 1],
                )

            # best (mark weights folded into column scales on host)
            nc.vector.tensor_sub(bt[:], bt[:], bgt[:])
            sq_acc(bgt[:], bt[:], stats[:BL, C_BEST : C_BEST + 1])

            # psum -> sbuf -> dram
            evac = cst.tile([128, 128], F32)
            nc.vector.tensor_copy(evac[:], cps[:])
            nc.sync.dma_start(out[:], stats[:])
            nc.sync.dma_start(out2[:], evac[:])

    nc.compile()
    return nc


def kernel(
    zs, rzs, pts, best, qy, gts, best_gt, mapping, vector_dims, **trace_kwargs
):
    global _module, last_results
    vd = int(np.asarray(vector_dims))
    assert vd == V, f"kernel compiled for vector_dims={V}, got {vd}"

    if _module is None:
        _module = _build_module()

    F8N = ml_dtypes.float8_e4m3
    BF = ml_dtypes.bfloat16
    mapping = np.asarray(mapping).astype(np.int32)
    qy = np.asarray(qy, dtype=np.float32).astype(BF)

    # point-column permutation: the 8 mark columns first
    rest = [i for i in range(P) if i not in MARK]
    perm = np.array(list(MARK) + rest)

    pts_p = np.asarray(pts, dtype=np.float32)[:, :, perm, :].reshape(B, S, P2)
    gts_p = np.asarray(gts, dtype=np.float32)[:, :, perm, :].reshape(B, SG, P2)
    zs = np.asarray(zs, dtype=np.float32)
    rzs = np.asarray(rzs, dtype=np.float32)

    comb = np.concatenate([rzs, pts_p], axis=2).astype(F8N)  # [B, S, CW]
    tgt = -np.concatenate([zs[:, :SG], gts_p], axis=2)  # [B, SG, CW] f32

    # best: fold the 10x landmark mse into column scales (f32, no overflow)
    wcol = np.ones(P2, np.float32)
    wcol[2 * np.array(MARK)] = MW
    wcol[2 * np.array(MARK) + 1] = MW
    best2 = np.asarray(best, dtype=np.float32).reshape(B, P2) * wcol
    bgt2 = np.asarray(best_gt, dtype=np.float32).reshape(B, P2) * wcol

    pp = np.arange(128)
    in_maps = []
    for c in range(NCORES):
        sl = slice(c * BL, (c + 1) * BL)
        mp = mapping[sl]  # [BL, SG]
        b = pp // 8
        pos = 16 * (pp % 8)[:, None] + np.arange(BL)[None, :]
        idx2 = (b[:, None] * S + mp[b[:, None], pos]).astype(np.int32)
        # tgt rows: [16b, 8g, 16k, CW] -> row (b*8+g)*16+k matches partition
        # p = b*8+g slot k, i.e. sample i = 16*g+k
        tgt_c = tgt[sl].reshape(BL, 8, BL, CW).astype(F8N)
        in_maps.append(
            {
                "comb": comb[sl].reshape(BL * S, CW),
                "tgt": np.ascontiguousarray(tgt_c).reshape(128 * BL, CW),
                "qy": qy[sl].reshape(BL * S, V),
                "best": np.ascontiguousarray(best2[sl]),
                "best_gt": np.ascontiguousarray(bgt2[sl]),
                "idx2": np.ascontiguousarray(idx2),
            }
        )

    last_results = run_bass_kernel_spmd(
        _module, in_maps, list(range(NCORES)), **trace_kwargs
    )
    tot = np.zeros(NSTAT, np.float64)
    tr = 0.0
    for r in last_results.results:
        tot += np.asarray(r["out"], dtype=np.float64).reshape(128, NSTAT).sum(axis=0)
        tr += np.trace(np.asarray(r["out2"], dtype=np.float64).reshape(128, 128))

    a_zs = tot[C_AZS : C_AZS + NCH].sum() + tot[C_VZS : C_VZS + NCH].sum() + tr
    ae_loss = a_zs / (B * SG * D)
    bias_loss = tot[C_PTS : C_PTS + NCH].sum() / (B * SG * P2) + ALPHA * tot[
        C_MRK : C_MRK + NCH
    ].sum() / (B * SG * NMARK2)
    kld_loss = tot[C_KLD] / (B * S)
    best_mse = tot[C_BEST] / (B * P2)

    return np.array(kld_loss + ae_loss + best_mse + bias_loss, dtype=np.float32)


# revision 7
# speedup vs baseline: 1.0096x; 1.0096x over previous
"""CQVAE loss kernel for Trainium2, data-parallel over batch on 8 NeuronCores.

loss = kld(qy) + mse(gather(rzs), zs[:, :Sg]) + bias(best, best_gt)
       + bias(gather(pts), gts)
where bias(p, g) = mse(p, g) + 10 * mse(p[..., MARK, :], g[..., MARK, :]).

Design (per core, 16 of 128 batches):
- Host packs comb rows [rzs | pts(mark-cols-first)] and a NEGATED target
  [zs | gts(mark-cols-first)], both fp8e4m3 (tolerance 2e-2 >> fp8 noise;
  rel err lands ~1e-3), halving HBM traffic vs bf16.
- The negated target direct-loads as 4 chunk tiles; the mapping-gather
  runs as 16 per-slot indirect DMAs with compute_op=add accumulating
  onto those tiles, so SBUF ends up holding d = comb_g - [zs|gts] and no
  vector subtract pass exists.  (The SWDGE CCE path is the only DMA
  accumulate the hardware supports, only with a [128,1] offset column
  and descriptors <= ~2KB, hence this exact shape.)
- d**2 + accumulate splits between Act (zs columns [0:ACOL), Square with
  accum) and DVE (rest of zs, pts, marks, kld, best); landmark weights
  use separate accumulators (marks permuted to the front of the pts
  block) or host-folded column scales (best), so no per-landmark
  reduction ops.
"""

import sys

import numpy as np

try:
    import concourse  # noqa: F401
except ImportError:  # pragma: no cover
    sys.path.insert(0, "/opt/trn_rl_repo")

import ml_dtypes

import concourse.bass as bass  # noqa: F401
import concourse.mybir as mybir
import concourse.tile as tile
from concourse import bacc
from concourse.bass_utils import run_bass_kernel_spmd

F32 = mybir.dt.float32
BF16 = mybir.dt.bfloat16
F8 = mybir.dt.float8e4
I32 = mybir.dt.int32
AX = mybir.AxisListType
OP = mybir.AluOpType
ACTF = mybir.ActivationFunctionType

NCORES = 8
B, S, SG, D, P, V = 128, 256, 128, 1024, 118, 64
BL = B // NCORES  # batches per core
P2 = 2 * P  # 236 point floats per row
CW = D + P2  # 1260 row width
MARK = (0, 29, 88, 117)
NMARK2 = 2 * len(MARK)  # 8 mark columns
EPS = 1e-20
ALPHA = 10.0

NCH = 4  # compute chunks
KC = BL // NCH  # 4 slots per chunk
CHW = KC * CW  # 5040 cols per chunk tile

ACOL = 652  # Act engine's zs column slice [0:ACOL); DVE takes the rest

QN = BL * S // 128  # 32 qy rows per partition
QCOLS = QN * V  # 2048

NSTAT = 20
C_AZS = 0  # 4 cols: Act zs accums
C_VZS = 4  # 4 cols: DVE zs accums
C_PTS = 8  # 4 cols: pts-all accums
C_MRK = 12  # 4 cols: pts mark accums
C_KLD = 16
C_BEST = 17

MW = float(np.sqrt(1.0 + ALPHA * P2 / NMARK2))  # 17.2047 best-mark fold

_module = None
last_results = None  # BassKernelResults of the most recent run (for profiling)


def _build_module():
    nc = bacc.Bacc()

    # comb row r=b*S+s : concat(rzs[b,s], pts_perm[b,s]) fp8
    comb = nc.dram_tensor("comb", [BL * S, CW], F8, kind="ExternalInput")
    # tgt row p*16+k : -concat(zs[b,i], gts_perm[b,i]), b=p//8, i=16*(p%8)+k
    tgt = nc.dram_tensor("tgt", [128 * BL, CW], F8, kind="ExternalInput")
    qy = nc.dram_tensor("qy", [BL * S, V], BF16, kind="ExternalInput")
    best = nc.dram_tensor("best", [BL, P2], F32, kind="ExternalInput")
    best_gt = nc.dram_tensor("best_gt", [BL, P2], F32, kind="ExternalInput")
    # idx[p, k] = (p//8)*S + mapping[p//8, 16*(p%8) + k]
    idx2 = nc.dram_tensor("idx2", [128, BL], I32, kind="ExternalInput")
    out = nc.dram_tensor("out", [128, NSTAT], F32, kind="ExternalOutput")

    with tile.TileContext(nc) as tc:
        with tc.tile_pool(name="cst", bufs=1) as cst:
            idx_t = cst.tile([128, BL], I32)
            nc.sync.dma_start(idx_t[:], idx2[:])

            stats = cst.tile([128, NSTAT], F32)
            nc.vector.memset(stats[:], 0.0)
            ebias = cst.tile([128, 1], F32)
            nc.vector.memset(ebias[:], float(V) * EPS)

            # negated target chunks, then qy/best, all on the sync queue
            tgt_r = tgt[:].rearrange("(p k) c -> p (k c)", k=BL)
            ch = []
            for c in range(NCH):
                t = cst.tile([128, CHW], F8, tag=f"ch{c}", name=f"ch{c}")
                nc.sync.dma_start(t[:], tgt_r[:, c * CHW : (c + 1) * CHW])
                ch.append(t)
            qy_t = cst.tile([128, QCOLS], BF16)
            nc.sync.dma_start(qy_t[:], qy[:].rearrange("(p n) v -> p (n v)", n=QN))
            bt = cst.tile([BL, P2], F32)
            nc.sync.dma_start(bt[:], best[:])
            bgt = cst.tile([BL, P2], F32)
            nc.sync.dma_start(bgt[:], best_gt[:])

            # gathers: per-slot indirect ops accumulating onto the negated
            # target -> each tile ends as comb_g - [zs|gts]
            for c in range(NCH):
                for k in range(KC):
                    nc.gpsimd.indirect_dma_start(
                        out=ch[c][:, k * CW : (k + 1) * CW],
                        out_offset=None,
                        in_=comb[:],
                        in_offset=bass.IndirectOffsetOnAxis(
                            ap=idx_t[:, c * KC + k : c * KC + k + 1], axis=0
                        ),
                        compute_op=OP.add,
                    )

            # ---- Act: Ln(qy) then zs column slice [0:ACOL) ----------------
            lg = cst.tile([128, QCOLS], BF16)
            nc.scalar.activation(lg[:], qy_t[:], ACTF.Ln, bias=ebias[:], scale=float(V))
            scr_a = cst.tile([128, KC * ACOL], BF16)
            sa3 = scr_a[:].rearrange("p (k w) -> p k w", w=ACOL)
            for c in range(NCH):
                c3 = ch[c][:].rearrange("p (k w) -> p k w", w=CW)
                nc.scalar.activation(
                    sa3, c3[:, :, 0:ACOL], ACTF.Square,
                    accum_out=stats[:, C_AZS + c : C_AZS + c + 1],
                )

            # ---- DVE: kld, zs tail, pts, marks, best ----------------------
            scr_k = cst.tile([128, QCOLS], BF16)
            scr_v = cst.tile([128, 2600], BF16)
            DVW = D - ACOL
            sv_z = scr_v[:, : KC * DVW].rearrange("p (k w) -> p k w", w=DVW)
            sv_p = scr_v[:, KC * DVW : KC * DVW + KC * P2].rearrange(
                "p (k w) -> p k w", w=P2
            )
            sv_m = scr_v[:, 2536 : 2536 + KC * NMARK2].rearrange(
                "p (k w) -> p k w", w=NMARK2
            )

            def sq_acc(out_ap, in_ap, acc):
                nc.vector.scalar_tensor_tensor(
                    out=out_ap, in0=in_ap, scalar=0.0, in1=in_ap,
                    op0=OP.subtract, op1=OP.mult, accum_out=acc,
                )

            # kld: sum q * ln(V*q + V*eps) = sum q*(ln q - ln(1/V))
            nc.vector.scalar_tensor_tensor(
                out=scr_k[:], in0=lg[:], scalar=0.0, in1=qy_t[:],
                op0=OP.subtract, op1=OP.mult,
                accum_out=stats[:, C_KLD : C_KLD + 1],
            )
            for c in range(NCH):
                c3 = ch[c][:].rearrange("p (k w) -> p k w", w=CW)
                sq_acc(sv_z, c3[:, :, ACOL:D], stats[:, C_VZS + c : C_VZS + c + 1])
                sq_acc(sv_p, c3[:, :, D:CW], stats[:, C_PTS + c : C_PTS + c + 1])
                sq_acc(
                    sv_m, c3[:, :, D : D + NMARK2],
                    stats[:, C_MRK + c : C_MRK + c + 1],
                )

            # best (mark weights folded into column scales on host)
            nc.vector.tensor_sub(bt[:], bt[:], bgt[:])
            sq_acc(bgt[:], bt[:], stats[:BL, C_BEST : C_BEST + 1])

            nc.sync.dma_start(out[:], stats[:])

    nc.compile()
    return nc


def kernel(
    zs, rzs, pts, best, qy, gts, best_gt, mapping, vector_dims, **trace_kwargs
):
    global _module, last_results
    vd = int(np.asarray(vector_dims))
    assert vd == V, f"kernel compiled for vector_dims={V}, got {vd}"

    if _module is None:
        _module = _build_module()

    F8N = ml_dtypes.float8_e4m3
    BF = ml_dtypes.bfloat16
    mapping = np.asarray(mapping).astype(np.int32)
    qy = np.asarray(qy, dtype=np.float32).astype(BF)

    # point-column permutation: the 8 mark columns first
    rest = [i for i in range(P) if i not in MARK]
    perm = np.array(list(MARK) + rest)

    pts_p = np.asarray(pts, dtype=np.float32)[:, :, perm, :].reshape(B, S, P2)
    gts_p = np.asarray(gts, dtype=np.float32)[:, :, perm, :].reshape(B, SG, P2)
    zs = np.asarray(zs, dtype=np.float32)
    rzs = np.asarray(rzs, dtype=np.float32)

    comb = np.concatenate([rzs, pts_p], axis=2).astype(F8N)  # [B, S, CW]
    tgt = -np.concatenate([zs[:, :SG], gts_p], axis=2)  # [B, SG, CW] f32

    # best: fold the 10x landmark mse into column scales (f32, no overflow)
    wcol = np.ones(P2, np.float32)
    wcol[2 * np.array(MARK)] = MW
    wcol[2 * np.array(MARK) + 1] = MW
    best2 = np.asarray(best, dtype=np.float32).reshape(B, P2) * wcol
    bgt2 = np.asarray(best_gt, dtype=np.float32).reshape(B, P2) * wcol

    pp = np.arange(128)
    b = pp // 8
    pos = 16 * (pp % 8)[:, None] + np.arange(BL)[None, :]
    in_maps = []
    for c in range(NCORES):
        sl = slice(c * BL, (c + 1) * BL)
        mp = mapping[sl]  # [BL, SG]
        idx2 = (b[:, None] * S + mp[b[:, None], pos]).astype(np.int32)
        # tgt rows: [16b, 8g, 16k, CW] -> row (b*8+g)*16+k = partition
        # p=b*8+g slot k, i.e. sample i = 16*g+k
        tgt_c = tgt[sl].reshape(BL, 8, BL, CW).astype(F8N)
        in_maps.append(
            {
                "comb": comb[sl].reshape(BL * S, CW),
                "tgt": np.ascontiguousarray(tgt_c).reshape(128 * BL, CW),
                "qy": qy[sl].reshape(BL * S, V),
                "best": np.ascontiguousarray(best2[sl]),
                "best_gt": np.ascontiguousarray(bgt2[sl]),
                "idx2": np.ascontiguousarray(idx2),
            }
        )

    last_results = run_bass_kernel_spmd(
        _module, in_maps, list(range(NCORES)), **trace_kwargs
    )
    tot = np.zeros(NSTAT, np.float64)
    for r in last_results.results:
        tot += np.asarray(r["out"], dtype=np.float64).reshape(128, NSTAT).sum(axis=0)

    a_zs = tot[C_AZS : C_AZS + NCH].sum() + tot[C_VZS : C_VZS + NCH].sum()
    ae_loss = a_zs / (B * SG * D)
    bias_loss = tot[C_PTS : C_PTS + NCH].sum() / (B * SG * P2) + ALPHA * tot[
        C_MRK : C_MRK + NCH
    ].sum() / (B * SG * NMARK2)
    kld_loss = tot[C_KLD] / (B * S)
    best_mse = tot[C_BEST] / (B * P2)

    return np.array(kld_loss + ae_loss + best_mse + bias_loss, dtype=np.float32)
